# revision 20
# baseline (speedup 1.0000x reference)
"""GAT layer (dense-softmax graph attention) on Trainium2, 8 NeuronCores.

Math (matches the reference exactly):
    Wh    = x @ W
    s_src = Wh @ a[:F_OUT] = x @ (W @ a[:F_OUT])
    s_dst = Wh @ a[F_OUT:] = x @ (W @ a[F_OUT:])
    e_ij  = leaky_relu(s_src[i] + s_dst[j], 0.2)
    att   = softmax_row(where(adj != 0, e, 0))
    out   = (att @ Wh).reshape(N, H, F_OUT/H).mean(axis=1)
          = att @ (x @ W_headmean)            # mean commutes with att @ .

Key identities used on device:
    p_ij = exp(adj_ij * lrelu(s_src_i + s_dst_j))   (non-edge -> exp(0) = 1,
           exactly the dense-softmax behaviour of the reference)
    row numerator+denominator in one matmul via a ones column:
           [h'_i | d_i] = sum_j p_ij * [Whm_j | 1]
    out_i = h'_i / d_i

Sharding: 1D partition of output rows i across 8 cores. Each core reads its
column slice adj[:, i_slice] (transposed layout: j on partitions, i on the
free dim) plus all of x (needed for the row-global s_dst / Whm), and writes
its own 1024 output rows. No cross-core communication.

Host-side prep (weight folding + layout marshalling only):
    B   = [W @ a_src | W @ a_dst | W.reshape(F_IN,H,FM).mean(1)]  [F_IN, 66]
    xT  = x.T (shared across cores), xsT = x[i_slice].T (per core)
    adjc = adj[:, i_slice] (per core)
"""

import numpy as np

import concourse.bass as bass
import concourse.bacc as bacc
import concourse.tile as tile
from concourse import mybir
from concourse.bass_utils import run_bass_kernel_spmd
from concourse.masks import make_identity

P = 128
F_IN = 512
F_OUT = 256
HEADS = 4
FM = F_OUT // HEADS        # 64 folded (head-averaged) features
FC = FM + 2                # 66 columns of B: [ws, wd, Wm]
YC = FM + 3                # 67 columns of a Y chunk: [s_src, s_dst, Whm, ones]
KC = F_IN // P             # 4 contraction chunks
N_CORES = 8
N_FULL = 8192
LRELU_SLOPE = 0.2


def build_nc(n=N_FULL, r=None, split_x_period=4, debug=False):
    """Build the SPMD Bass program (same program on every core).

    n: total number of graph nodes; r: output rows per core.
    split_x_period: every split_x_period-th j-tile uses the DVE-heavy
    elementwise split instead of the ACT-heavy one (engine balancing).
    """
    if r is None:
        r = n // N_CORES
    assert n % P == 0 and r % P == 0
    jt_n = n // P              # number of 128-row j-chunks
    ibw = min(512, n)          # xT i-block width for the Y precompute
    nib = n // ibw
    jcb = ibw // P             # y-chunks per i-block
    ab = min(8, jt_n)          # adj j-tiles per DMA batch
    n_ab = jt_n // ab
    mov = min(r, 512)          # moving free-dim per matmul (fp32 limit 512)
    mh = r // mov
    ich = r // P               # output row chunks
    f32 = mybir.dt.float32
    f32r = mybir.dt.float32r
    i32 = mybir.dt.int32
    AF = mybir.ActivationFunctionType
    OP = mybir.AluOpType

    nc = bacc.Bacc(None, target_bir_lowering=False)
    xT_d = nc.dram_tensor("xT", [F_IN, n], f32, kind="ExternalInput")
    xsT_d = nc.dram_tensor("xsT", [F_IN, r], f32, kind="ExternalInput")
    adj_d = nc.dram_tensor("adjc", [n, r], i32, kind="ExternalInput")
    B_d = nc.dram_tensor("B", [F_IN, FC], f32, kind="ExternalInput")
    h_d = nc.dram_tensor("h", [r, FM], f32, kind="ExternalOutput")
    if debug:
        dbg_ssrc = nc.dram_tensor("dbg_ssrc", [P, r], f32, kind="ExternalOutput")
        dbg_y0 = nc.dram_tensor("dbg_y0", [P, YC], f32, kind="ExternalOutput")
        dbg_y1 = nc.dram_tensor("dbg_y1", [P, YC], f32, kind="ExternalOutput")
        dbg_u0 = nc.dram_tensor("dbg_u0", [P, r], f32, kind="ExternalOutput")
        dbg_p0 = nc.dram_tensor("dbg_p0", [P, r], f32, kind="ExternalOutput")
        dbg_acc = nc.dram_tensor("dbg_acc", [FM + 1, r], f32, kind="ExternalOutput")

    with tile.TileContext(nc) as tc:
        with (
            tc.tile_pool(name="consts", bufs=1) as consts,
            tc.tile_pool(name="ypool", bufs=jt_n) as ypool,
            tc.tile_pool(name="xpool", bufs=2) as xpool,
            tc.tile_pool(name="adjpool", bufs=2) as adjpool,
            tc.tile_pool(name="upool", bufs=2) as upool,
            tc.tile_pool(name="tpool", bufs=2) as tpool,
            tc.tile_pool(name="ppool", bufs=3) as ppool,
            tc.tile_pool(name="mpool", bufs=2) as mpool,
            tc.tile_pool(name="yps", bufs=2, space="PSUM") as yps,
            tc.tile_pool(name="sps", bufs=1, space="PSUM") as sps,
            tc.tile_pool(name="accps", bufs=1, space="PSUM") as accps,
            tc.tile_pool(name="tailps", bufs=2, space="PSUM") as tailps,
        ):
            # ---- constants ----
            b_sb = consts.tile([P, KC, FC], f32)
            nc.scalar.dma_start(b_sb[:], B_d.rearrange("(kc p) f -> p kc f", p=P))
            ident = consts.tile([P, P], f32)
            make_identity(nc, ident)

            # ---- s_src broadcast [P, r]: ones(P) outer s_src(i_slice) ----
            # stationary wsb[k, m] = ws[k] for every m, so the matmul output
            # row m is s_src for all partitions m simultaneously.
            xst = consts.tile([P, KC, r], f32)
            nc.scalar.dma_start(xst[:], xsT_d.rearrange("(kc p) i -> p kc i", p=P))
            wsb = consts.tile([P, KC, P], f32)
            for kc in range(KC):
                nc.vector.tensor_copy(
                    wsb[:, kc, :], b_sb[:, kc, 0:1].to_broadcast([P, P])
                )
            ssb_ps = sps.tile([P, r], f32)
            for kc in range(KC):
                for hh in range(mh):
                    nc.tensor.matmul(
                        ssb_ps[:, hh * mov:(hh + 1) * mov],
                        wsb[:, kc, :],
                        xst[:, kc, hh * mov:(hh + 1) * mov],
                        start=(kc == 0),
                        stop=(kc == KC - 1),
                    )
            s_src = consts.tile([P, r], f32)
            nc.vector.tensor_copy(s_src[:], ssb_ps[:])

            # ---- Y chunks: Y[jc] = [s_src | s_dst | Whm | 1] per 128 rows ----
            ytiles = []
            xT_r = xT_d.rearrange("(kc p) i -> p kc i", p=P)
            for ib in range(nib):
                xt = xpool.tile([P, KC, ibw], f32, tag="xt")
                nc.scalar.dma_start(
                    xt[:], xT_r[:, :, ib * ibw:(ib + 1) * ibw]
                )
                for jl in range(jcb):
                    y_ps = yps.tile([P, FC], f32, tag="yps")
                    for kc in range(KC):
                        nc.tensor.matmul(
                            y_ps[:],
                            xt[:, kc, jl * P:(jl + 1) * P],
                            b_sb[:, kc, :],
                            start=(kc == 0),
                            stop=(kc == KC - 1),
                        )
                    # s-columns stay fp32 (used as ACT bias / stt scalar);
                    # Whm+ones live in a separate fp32r tile because the
                    # accumulation matmul consumes them as fp32r stationary
                    # and the BIR verifier tracks rounding per memory location.
                    ys = ypool.tile([P, 2], f32, tag="ys")
                    nc.vector.tensor_copy(ys[:], y_ps[:, 0:2])
                    yw = ypool.tile([P, FM + 1], f32r, tag="yw")
                    nc.vector.tensor_copy(yw[:, 0:FM], y_ps[:, 2:FC])
                    # ones column (softmax denominator row of the matmul);
                    # memset can't encode f32r, so write it as in*0 + 1
                    nc.vector.tensor_scalar(
                        out=yw[:, FM:FM + 1], in0=y_ps[:, 0:1],
                        scalar1=0.0, scalar2=1.0,
                        op0=OP.mult, op1=OP.add,
                    )
                    ytiles.append((ys, yw))

            # ---- main pass over j-tiles ----
            acc = accps.tile([FM + 1, r], f32)
            adj_r = adj_d.rearrange("(b f p) i -> b p f i", f=ab, p=P)
            for b in range(n_ab):
                adjt = adjpool.tile([P, ab, r], i32, tag="adj")
                nc.sync.dma_start(adjt[:], adj_r[b])
                for f in range(ab):
                    jt = b * ab + f
                    ys, yw = ytiles[jt]
                    if (jt % split_x_period) != (split_x_period - 1):
                        # ACT-heavy split: ACT does bias-add + lrelu fused.
                        t = tpool.tile([P, r], f32, tag="t")
                        nc.scalar.activation(
                            t[:], s_src[:], AF.Prelu,
                            bias=ys[:, 1:2], scale=1.0, alpha=LRELU_SLOPE,
                        )
                        u = upool.tile([P, r], f32, tag="u")
                        nc.vector.scalar_tensor_tensor(
                            out=u[:], in0=t[:], scalar=1.0, in1=adjt[:, f, :],
                            op0=OP.mult, op1=OP.mult,
                        )
                    else:
                        # DVE-heavy split: stt does bias-add + mask, then
                        # lrelu(z) = max(0.2 z, z) as a second stt.
                        zu = upool.tile([P, r], f32, tag="u")
                        nc.vector.scalar_tensor_tensor(
                            out=zu[:], in0=s_src[:], scalar=ys[:, 1:2],
                            in1=adjt[:, f, :], op0=OP.add, op1=OP.mult,
                        )
                        u = tpool.tile([P, r], f32, tag="t")
                        nc.vector.scalar_tensor_tensor(
                            out=u[:], in0=zu[:], scalar=LRELU_SLOPE, in1=zu[:],
                            op0=OP.mult, op1=OP.max,
                        )
                    p = ppool.tile([P, r], f32r, tag="p")
                    nc.scalar.activation(p[:], u[:], AF.Exp)
                    if debug and jt == 0:
                        nc.gpsimd.dma_start(dbg_u0[:], u[:])
                        nc.gpsimd.dma_start(dbg_p0[:], p[:].bitcast(f32))
                    for hh in range(mh):
                        nc.tensor.matmul(
                            acc[:, hh * mov:(hh + 1) * mov],
                            yw[:],
                            p[:, hh * mov:(hh + 1) * mov],
                            start=(jt == 0),
                            stop=(jt == jt_n - 1),
                        )

            if debug:
                nc.gpsimd.dma_start(dbg_ssrc[:], s_src[:])
                nc.gpsimd.dma_start(dbg_y0[:, 0:2], ytiles[0][0][:])
                nc.gpsimd.dma_start(dbg_y0[:, 2:YC], ytiles[0][1][:].bitcast(f32))
                nc.gpsimd.dma_start(dbg_y1[:, 0:2], ytiles[1][0][:])
                nc.gpsimd.dma_start(dbg_y1[:, 2:YC], ytiles[1][1][:].bitcast(f32))

            # ---- tail: transpose [65, r] -> [r, 65], divide, store ----
            acc_sb = consts.tile([P, r], f32)
            nc.gpsimd.memset(acc_sb[FM:P, :], 0.0)
            nc.vector.tensor_copy(acc_sb[0:FM + 1, :], acc[:])
            if debug:
                nc.gpsimd.dma_start(dbg_acc[:], acc_sb[0:FM + 1, :])
            out_sb = consts.tile([P, ich, FM], f32)
            for ic in range(ich):
                tp = tailps.tile([P, P], f32, tag="tp")
                nc.tensor.transpose(
                    tp[:], acc_sb[:, ic * P:(ic + 1) * P], ident[:]
                )
                rec = mpool.tile([P, 1], f32, tag="rec")
                nc.vector.reciprocal(rec[:], tp[:, FM:FM + 1])
                nc.vector.tensor_scalar_mul(out_sb[:, ic, :], tp[:, 0:FM], rec[:])
            nc.sync.dma_start(h_d.rearrange("(c p) f -> p c f", p=P), out_sb[:])

    return nc


def fold_weights(W, a):
    """Host-side weight folding: B = [W@a_src | W@a_dst | head-mean(W)]."""
    W = np.asarray(W, dtype=np.float32)
    a = np.asarray(a, dtype=np.float32).reshape(2 * F_OUT)
    ws = W @ a[:F_OUT]                                   # [F_IN]
    wd = W @ a[F_OUT:]                                   # [F_IN]
    Wm = W.reshape(F_IN, HEADS, FM).mean(axis=1)         # [F_IN, FM]
    return np.ascontiguousarray(
        np.concatenate([ws[:, None], wd[:, None], Wm], axis=1), dtype=np.float32
    )


def shard_inputs(x, adj, W, a, n_cores=N_CORES):
    """Build the per-core input maps."""
    x = np.asarray(x, dtype=np.float32)
    adj = np.ascontiguousarray(np.asarray(adj), dtype=np.int32)
    n = x.shape[0]
    r = n // n_cores
    B = fold_weights(W, a)
    xT = np.ascontiguousarray(x.T)                       # [F_IN, n], shared
    in_maps = []
    for c in range(n_cores):
        i0 = c * r
        in_maps.append({
            "xT": xT,
            "xsT": np.ascontiguousarray(xT[:, i0:i0 + r]),
            # device layout is [j (partitions), i (free)] and the attention
            # mask for output row i, summed index j is adj[i, j] -> transpose
            "adjc": np.ascontiguousarray(adj[i0:i0 + r, :].T),
            "B": B,
        })
    return in_maps


def run(x, adj, W, a, n=N_FULL, trace=False, split_x_period=4):
    nc = build_nc(n=n)
    if not nc.is_finalized():
        nc.finalize()
    in_maps = shard_inputs(x, adj, W, a)
    core_ids = list(range(N_CORES))
    res = run_bass_kernel_spmd(nc, in_maps, core_ids, trace=trace)
    h = np.concatenate([res.results[c]["h"] for c in range(N_CORES)], axis=0)
    return h, res


def kernel(x, adj, W, a, heads=HEADS, **_ignored):
    assert int(heads) == HEADS, f"kernel hardcodes heads={HEADS}"
    assert x.shape == (N_FULL, F_IN) and adj.shape == (N_FULL, N_FULL)
    h, _ = run(x, adj, W, a, n=N_FULL, trace=False)
    return h.astype(np.float32)


# revision 22
# speedup vs baseline: 1.1810x; 1.1810x over previous
"""GAT layer (dense-softmax graph attention) on Trainium2, 8 NeuronCores.

Math (matches the reference exactly):
    Wh    = x @ W
    s_src = Wh @ a[:F_OUT] = x @ (W @ a[:F_OUT])
    s_dst = Wh @ a[F_OUT:] = x @ (W @ a[F_OUT:])
    e_ij  = leaky_relu(s_src[i] + s_dst[j], 0.2)
    att   = softmax_row(where(adj != 0, e, 0))
    out   = (att @ Wh).reshape(N, H, F_OUT/H).mean(axis=1)
          = att @ (x @ W_headmean)            # mean commutes with att @ .

Key identities used on device:
    p_ij = exp(adj_ij * lrelu(s_src_i + s_dst_j))   (non-edge -> exp(0) = 1,
           exactly the dense-softmax behaviour of the reference)
    row numerator+denominator in one matmul via a ones column:
           [h'_i | d_i] = sum_j p_ij * [Whm_j | 1]
    out_i = h'_i / d_i

Sharding: 1D partition of output rows i across 8 cores. Each core reads its
column slice adj[:, i_slice] (transposed layout: j on partitions, i on the
free dim) plus all of x (needed for the row-global s_dst / Whm), and writes
its own 1024 output rows. No cross-core communication.

Host-side prep (weight folding + layout marshalling only):
    B   = [W @ a_src | W @ a_dst | W.reshape(F_IN,H,FM).mean(1)]  [F_IN, 66]
    xT  = x.T (shared across cores), xsT = x[i_slice].T (per core)
    adjc = adj[:, i_slice] (per core)
"""

import numpy as np

import concourse.bass as bass
import concourse.bacc as bacc
import concourse.tile as tile
from concourse import mybir
from concourse.bass_utils import run_bass_kernel_spmd
from concourse.masks import make_identity

P = 128
F_IN = 512
F_OUT = 256
HEADS = 4
FM = F_OUT // HEADS        # 64 folded (head-averaged) features
FC = FM + 2                # 66 columns of B: [ws, wd, Wm]
YC = FM + 3                # 67 columns of a Y chunk: [s_src, s_dst, Whm, ones]
KC = F_IN // P             # 4 contraction chunks
N_CORES = 8
N_FULL = 8192
LRELU_SLOPE = 0.2


def build_nc(n=N_FULL, r=None, split_x_period=4, debug=False):
    """Build the SPMD Bass program (same program on every core).

    n: total number of graph nodes; r: output rows per core.
    split_x_period: every split_x_period-th j-tile uses the DVE-heavy
    elementwise split instead of the ACT-heavy one (engine balancing).
    """
    if r is None:
        r = n // N_CORES
    assert n % P == 0 and r % P == 0
    jt_n = n // P              # number of 128-row j-chunks
    ibw = min(512, n)          # xT i-block width for the Y precompute
    nib = n // ibw
    jcb = ibw // P             # y-chunks per i-block
    ab = min(8, jt_n)          # adj j-tiles per DMA batch
    n_ab = jt_n // ab
    mov = min(r, 512)          # moving free-dim per matmul (fp32 limit 512)
    mh = r // mov
    ich = r // P               # output row chunks
    f32 = mybir.dt.float32
    f32r = mybir.dt.float32r
    i32 = mybir.dt.int32
    AF = mybir.ActivationFunctionType
    OP = mybir.AluOpType

    nc = bacc.Bacc(None, target_bir_lowering=False)
    xT_d = nc.dram_tensor("xT", [F_IN, n], f32, kind="ExternalInput")
    xsT_d = nc.dram_tensor("xsT", [F_IN, r], f32, kind="ExternalInput")
    adj_d = nc.dram_tensor("adjc", [n, r], i32, kind="ExternalInput")
    B_d = nc.dram_tensor("B", [F_IN, FC], f32, kind="ExternalInput")
    h_d = nc.dram_tensor("h", [r, FM], f32, kind="ExternalOutput")
    if debug:
        dbg_ssrc = nc.dram_tensor("dbg_ssrc", [P, r], f32, kind="ExternalOutput")
        dbg_y0 = nc.dram_tensor("dbg_y0", [P, YC], f32, kind="ExternalOutput")
        dbg_y1 = nc.dram_tensor("dbg_y1", [P, YC], f32, kind="ExternalOutput")
        dbg_u0 = nc.dram_tensor("dbg_u0", [P, r], f32, kind="ExternalOutput")
        dbg_p0 = nc.dram_tensor("dbg_p0", [P, r], f32, kind="ExternalOutput")
        dbg_acc = nc.dram_tensor("dbg_acc", [FM + 1, r], f32, kind="ExternalOutput")

    with tile.TileContext(nc) as tc:
        with (
            tc.tile_pool(name="consts", bufs=1) as consts,
            tc.tile_pool(name="ypool", bufs=jt_n) as ypool,
            tc.tile_pool(name="xpool", bufs=2) as xpool,
            tc.tile_pool(name="adjpool", bufs=2) as adjpool,
            tc.tile_pool(name="upool", bufs=2) as upool,
            tc.tile_pool(name="tpool", bufs=2) as tpool,
            tc.tile_pool(name="ppool", bufs=3) as ppool,
            tc.tile_pool(name="mpool", bufs=2) as mpool,
            tc.tile_pool(name="yps", bufs=2, space="PSUM") as yps,
            tc.tile_pool(name="sps", bufs=1, space="PSUM") as sps,
            tc.tile_pool(name="accps", bufs=1, space="PSUM") as accps,
            tc.tile_pool(name="tailps", bufs=2, space="PSUM") as tailps,
        ):
            # ---- constants ----
            b_sb = consts.tile([P, KC, FC], f32)
            nc.scalar.dma_start(b_sb[:], B_d.rearrange("(kc p) f -> p kc f", p=P))
            ident = consts.tile([P, P], f32)
            make_identity(nc, ident)

            # ---- s_src broadcast [P, r]: ones(P) outer s_src(i_slice) ----
            # stationary wsb[k, m] = ws[k] for every m, so the matmul output
            # row m is s_src for all partitions m simultaneously.
            xst = consts.tile([P, KC, r], f32)
            nc.scalar.dma_start(xst[:], xsT_d.rearrange("(kc p) i -> p kc i", p=P))
            wsb = consts.tile([P, KC, P], f32)
            for kc in range(KC):
                nc.vector.tensor_copy(
                    wsb[:, kc, :], b_sb[:, kc, 0:1].to_broadcast([P, P])
                )
            ssb_ps = sps.tile([P, r], f32)
            for kc in range(KC):
                for hh in range(mh):
                    nc.tensor.matmul(
                        ssb_ps[:, hh * mov:(hh + 1) * mov],
                        wsb[:, kc, :],
                        xst[:, kc, hh * mov:(hh + 1) * mov],
                        start=(kc == 0),
                        stop=(kc == KC - 1),
                    )
            s_src = consts.tile([P, r], f32)
            nc.vector.tensor_copy(s_src[:], ssb_ps[:])

            # ---- Y chunks: Y[jc] = [s_src | s_dst | Whm | 1] per 128 rows ----
            # Computed transposed (stationary = B chunks, moving = xT at
            # N=512) so the PE streams at full width instead of paying an
            # unhidden LDWEIGHTS per 66-column matmul, then PE-transposed
            # back to row-chunk layout. Row 66 of the transpose input is
            # pinned to 1.0, which lands the softmax-denominator ones
            # column in the transposed output for free.
            ytiles = []
            xT_r = xT_d.rearrange("(kc p) i -> p kc i", p=P)
            for ib in range(nib):
                xt = xpool.tile([P, KC, ibw], f32, tag="xt")
                nc.scalar.dma_start(
                    xt[:], xT_r[:, :, ib * ibw:(ib + 1) * ibw]
                )
                yt_ps = yps.tile([FC, ibw], f32, tag="yps")
                for kc in range(KC):
                    nc.tensor.matmul(
                        yt_ps[:],
                        b_sb[:, kc, :],
                        xt[:, kc, :],
                        start=(kc == 0),
                        stop=(kc == KC - 1),
                    )
                ytb = xpool.tile([P, ibw], f32, tag="ytb")
                nc.gpsimd.memset(ytb[FM:P, :], 0.0)
                nc.vector.tensor_copy(ytb[0:FC, :], yt_ps[:])
                for jl in range(jcb):
                    tp = tailps.tile([P, P], f32, tag="tp")
                    nc.tensor.transpose(
                        tp[:], ytb[:, jl * P:(jl + 1) * P], ident[:]
                    )
                    # s-columns stay fp32 (ACT bias / stt scalar); Whm+ones
                    # go to a separate fp32r tile (fp32r stationary of the
                    # accumulation matmul; rounding tracked per location).
                    ys = ypool.tile([P, 2], f32, tag="ys")
                    nc.vector.tensor_copy(ys[:], tp[:, 0:2])
                    yw = ypool.tile([P, FM + 1], f32r, tag="yw")
                    nc.vector.tensor_copy(yw[:, 0:FM], tp[:, 2:FC])
                    nc.vector.tensor_scalar(
                        out=yw[:, FM:FM + 1], in0=ys[:, 0:1],
                        scalar1=0.0, scalar2=1.0,
                        op0=OP.mult, op1=OP.add,
                    )
                    ytiles.append((ys, yw))

            # ---- main pass over j-tiles ----
            acc = accps.tile([FM + 1, r], f32)
            adj_r = adj_d.rearrange("(b f p) i -> b p f i", f=ab, p=P)
            for b in range(n_ab):
                adjt = adjpool.tile([P, ab, r], i32, tag="adj")
                nc.sync.dma_start(adjt[:], adj_r[b])
                for f in range(ab):
                    jt = b * ab + f
                    ys, yw = ytiles[jt]
                    if (jt % split_x_period) != (split_x_period - 1):
                        # ACT-heavy split: ACT does bias-add + lrelu fused.
                        t = tpool.tile([P, r], f32, tag="t")
                        nc.scalar.activation(
                            t[:], s_src[:], AF.Prelu,
                            bias=ys[:, 1:2], scale=1.0, alpha=LRELU_SLOPE,
                        )
                        u = upool.tile([P, r], f32, tag="u")
                        nc.vector.scalar_tensor_tensor(
                            out=u[:], in0=t[:], scalar=1.0, in1=adjt[:, f, :],
                            op0=OP.mult, op1=OP.mult,
                        )
                    else:
                        # DVE-heavy split: stt does bias-add + mask, then
                        # lrelu(z) = max(0.2 z, z) as a second stt.
                        zu = upool.tile([P, r], f32, tag="u")
                        nc.vector.scalar_tensor_tensor(
                            out=zu[:], in0=s_src[:], scalar=ys[:, 1:2],
                            in1=adjt[:, f, :], op0=OP.add, op1=OP.mult,
                        )
                        u = tpool.tile([P, r], f32, tag="t")
                        nc.vector.scalar_tensor_tensor(
                            out=u[:], in0=zu[:], scalar=LRELU_SLOPE, in1=zu[:],
                            op0=OP.mult, op1=OP.max,
                        )
                    p = ppool.tile([P, r], f32r, tag="p")
                    nc.scalar.activation(p[:], u[:], AF.Exp)
                    if debug and jt == 0:
                        nc.gpsimd.dma_start(dbg_u0[:], u[:])
                        nc.gpsimd.dma_start(dbg_p0[:], p[:].bitcast(f32))
                    for hh in range(mh):
                        nc.tensor.matmul(
                            acc[:, hh * mov:(hh + 1) * mov],
                            yw[:],
                            p[:, hh * mov:(hh + 1) * mov],
                            start=(jt == 0),
                            stop=(jt == jt_n - 1),
                        )

            if debug:
                nc.gpsimd.dma_start(dbg_ssrc[:], s_src[:])
                nc.gpsimd.dma_start(dbg_y0[:, 0:2], ytiles[0][0][:])
                nc.gpsimd.dma_start(dbg_y0[:, 2:YC], ytiles[0][1][:].bitcast(f32))
                nc.gpsimd.dma_start(dbg_y1[:, 0:2], ytiles[1][0][:])
                nc.gpsimd.dma_start(dbg_y1[:, 2:YC], ytiles[1][1][:].bitcast(f32))

            # ---- tail: transpose [65, r] -> [r, 65], divide, store ----
            acc_sb = consts.tile([P, r], f32)
            nc.gpsimd.memset(acc_sb[FM:P, :], 0.0)
            nc.vector.tensor_copy(acc_sb[0:FM + 1, :], acc[:])
            if debug:
                nc.gpsimd.dma_start(dbg_acc[:], acc_sb[0:FM + 1, :])
            out_sb = consts.tile([P, ich, FM], f32)
            for ic in range(ich):
                tp = tailps.tile([P, P], f32, tag="tp")
                nc.tensor.transpose(
                    tp[:], acc_sb[:, ic * P:(ic + 1) * P], ident[:]
                )
                rec = mpool.tile([P, 1], f32, tag="rec")
                nc.vector.reciprocal(rec[:], tp[:, FM:FM + 1])
                nc.vector.tensor_scalar_mul(out_sb[:, ic, :], tp[:, 0:FM], rec[:])
            nc.sync.dma_start(h_d.rearrange("(c p) f -> p c f", p=P), out_sb[:])

    return nc


def fold_weights(W, a):
    """Host-side weight folding: B = [W@a_src | W@a_dst | head-mean(W)]."""
    W = np.asarray(W, dtype=np.float32)
    a = np.asarray(a, dtype=np.float32).reshape(2 * F_OUT)
    ws = W @ a[:F_OUT]                                   # [F_IN]
    wd = W @ a[F_OUT:]                                   # [F_IN]
    Wm = W.reshape(F_IN, HEADS, FM).mean(axis=1)         # [F_IN, FM]
    return np.ascontiguousarray(
        np.concatenate([ws[:, None], wd[:, None], Wm], axis=1), dtype=np.float32
    )


def shard_inputs(x, adj, W, a, n_cores=N_CORES):
    """Build the per-core input maps."""
    x = np.asarray(x, dtype=np.float32)
    adj = np.ascontiguousarray(np.asarray(adj), dtype=np.int32)
    n = x.shape[0]
    r = n // n_cores
    B = fold_weights(W, a)
    xT = np.ascontiguousarray(x.T)                       # [F_IN, n], shared
    in_maps = []
    for c in range(n_cores):
        i0 = c * r
        in_maps.append({
            "xT": xT,
            "xsT": np.ascontiguousarray(xT[:, i0:i0 + r]),
            # device layout is [j (partitions), i (free)] and the attention
            # mask for output row i, summed index j is adj[i, j] -> transpose
            "adjc": np.ascontiguousarray(adj[i0:i0 + r, :].T),
            "B": B,
        })
    return in_maps


def run(x, adj, W, a, n=N_FULL, trace=False, split_x_period=4):
    nc = build_nc(n=n)
    if not nc.is_finalized():
        nc.finalize()
    in_maps = shard_inputs(x, adj, W, a)
    core_ids = list(range(N_CORES))
    res = run_bass_kernel_spmd(nc, in_maps, core_ids, trace=trace)
    h = np.concatenate([res.results[c]["h"] for c in range(N_CORES)], axis=0)
    return h, res


def kernel(x, adj, W, a, heads=HEADS, **_ignored):
    assert int(heads) == HEADS, f"kernel hardcodes heads={HEADS}"
    assert x.shape == (N_FULL, F_IN) and adj.shape == (N_FULL, N_FULL)
    h, _ = run(x, adj, W, a, n=N_FULL, trace=False)
    return h.astype(np.float32)
